# revision 34
# baseline (speedup 1.0000x reference)
"""DenseValueWindowedPartialLM kernel for 8 trn2 NeuronCores.

Sharding: token-parallel.  The 4096 tokens (t = s*B + b) are split into 8
contiguous slices of 512; each core computes the FULL 32000-column logits
for its slice.  Per-core device work is exactly total_work/8 (~22 GFLOP).

Precision strategy (validated on host against the fp32 reference):
the final output is dominated by the scattered partial (absmax ~1.5e-2)
while base_logits are tiny (absmax ~4e-5), so the whole GEMM chain runs
in fp8e4m3 with DoubleRow perf mode (2x PE rate) and host-computed
per-tensor scales:
  A: hf = relu(wfc@s)^2, B: bf = whp@hf,
  C1: logits = bf.T@Wc, C2: base_partial = bf.T@wph.
The precision-critical u-term of the partial, u@wph.T (u = gate *
mem_scale * ctx is host-known), is added exactly on the host in fp32,
so device fp8 only ever touches the tiny base quantities.  Both device
outputs are written as scaled fp8, halving the output DMA.

The GRU scan (sequential, 2048 steps) and the tiny windowed attention
run on host; the untied-token scatter-add of the partial logits is a
vectorized host post-pass.

Perf notes (trace-driven; the PE runs gap-free at the fp8 ingest floor
of ~216ns per 512-column DoubleRow matmul):
- vocab is processed in 4096-column chunks (one 2MB weight DMA + 4
  output DMAs per chunk) to keep the sync engine's ~0.6us/descriptor
  issue rate off the critical path; chunk sizes taper at the end so the
  final drain+flush tail stays short.
- output DMAs issue from the otherwise idle gpsimd queue (the tail ones
  alternate with sync); C2 runs LAST so its compute overlaps the outL
  flush and only the small fp8 outP flush trails the last matmul.
- all PSUM goes through one [128,1024] bufs=4 pool (8 banks);
  PSUM->SBUF drains alternate between Vector and Scalar, which are the
  co-bottleneck next to the PE.
- critical-path input loads (sT halves, wfc quarters) are split across
  the sync+scalar queues; 12 warmup matmuls ramp the PE clock while
  they land.
"""

import sys

sys.path.insert(0, "/opt/trn_rl_repo")

import numpy as np

try:
    import concourse.bass as bass
    import concourse.bacc as bacc
    import concourse.mybir as mybir
    import concourse.tile as tile
    from concourse.bass_utils import run_bass_kernel_spmd
    _HAVE_BASS = True
except Exception:  # toolchain unavailable -> host fallback only
    _HAVE_BASS = False

B, S, V, E, H, MD, P, W = 2, 2048, 32000, 512, 1024, 256, 4096, 128
FF = 4 * E                 # 2048
NCORES = 8
TOK = B * S                # 4096 tokens, row t = s*B + b
TPC = TOK // NCORES        # 512 tokens per core
KH, KF, KE = H // 128, FF // 128, E // 128   # 8, 16, 4
NT = TPC // 128                              # 4 token blocks
CH = 4096                                    # vocab chunk
# 7x4096 then tapering tail chunks: the final output DMAs are issued at
# ~0.64us each from the gpsimd queue, so taper chunk sizes to keep the
# end-of-kernel drain+DMA tail short.
CHUNKS = [(b0, CH) for b0 in range(0, 7 * CH, CH)] + [
    (28672, 1024), (29696, 1024), (30720, 768), (31488, 512)]
M8 = 192.0                                   # fp8 target absmax (max norm 240)
if _HAVE_BASS:
    BF = mybir.dt.bfloat16
    F32 = mybir.dt.float32
    FP8 = mybir.dt.float8e4
    AF = mybir.ActivationFunctionType
    DR = mybir.MatmulPerfMode.DoubleRow

_cached = {}

# scale-vector column indices
SC_ALPHA, SC_BETA, SC_OSC2, SC_OSC1 = 0, 1, 2, 3


def _build_program():
    """Per-core program, token slice of TPC=512 tokens (N dim of matmuls).

    A  (fp8 DR): hfT[FF,T] = relu(wfc.T @ sT + bfc)^2      (feature-major)
    B  (fp8 DR): bf8[E,T]  = (whp.T @ hfT + bhp') * beta
    C2 (fp8 DR): outP[T,P] = bf8.T @ wph8 * osc2           (token-major out)
    C1 (fp8 DR): outL[T,V] = bf8.T @ Wc8  * osc1 -> fp8    (token-major out)
    """
    nc = bacc.Bacc()
    d_sT = nc.dram_tensor("sT", [128, KH * TPC], FP8, kind="ExternalInput")
    d_wfc = nc.dram_tensor("wfc", [128, KF * KH * 128], FP8,
                           kind="ExternalInput")   # [p, (m, k, c)]
    d_whp = nc.dram_tensor("whp", [128, KE * KF * 128], FP8,
                           kind="ExternalInput")   # [p, (m, k, c)]
    d_wph = nc.dram_tensor("wph", [128, KE * P], FP8,
                           kind="ExternalInput")   # [p, (k, col)]
    d_Wc = nc.dram_tensor("Wc", [128, KE * V], FP8,
                          kind="ExternalInput")    # [p, (chunk, k, col)]
    d_bfc = nc.dram_tensor("bfc", [128, KF], F32, kind="ExternalInput")
    d_bhp = nc.dram_tensor("bhp", [128, KE], F32, kind="ExternalInput")
    d_scl = nc.dram_tensor("scl", [128, 4], F32, kind="ExternalInput")
    d_outL = nc.dram_tensor("outL", [TPC, V], FP8, kind="ExternalOutput")
    d_outP = nc.dram_tensor("outP", [TPC, P], FP8, kind="ExternalOutput")

    dcount = 0

    def drain_mul(dst, src, scol):
        nonlocal dcount
        if dcount % 2 == 0:
            nc.vector.tensor_scalar_mul(dst, src, scol)
        else:
            nc.scalar.mul(dst, src, scol)
        dcount += 1

    ocount = 0

    def out_dma(dst, src, spread=0):
        nonlocal ocount
        # tail chunks: spread issues over otherwise-idle queues so the
        # final flush isn't serialized on gpsimd's ~0.64us issue rate
        if spread and ocount % 2:
            eng = nc.sync
        else:
            eng = nc.gpsimd
        ocount += 1
        eng.dma_start(dst, src)

    with tile.TileContext(nc) as tc:
        with tc.tile_pool(name="sa", bufs=1) as p_s, \
             tc.tile_pool(name="wfc", bufs=1) as p_wfc, \
             tc.tile_pool(name="bias", bufs=1) as p_b, \
             tc.tile_pool(name="rl", bufs=3) as p_rl, \
             tc.tile_pool(name="hf", bufs=1) as p_hf, \
             tc.tile_pool(name="whp", bufs=1) as p_whp, \
             tc.tile_pool(name="bfu", bufs=1) as p_bf, \
             tc.tile_pool(name="wph", bufs=1) as p_wph, \
             tc.tile_pool(name="pbuf", bufs=6) as p_pb, \
             tc.tile_pool(name="wc", bufs=3) as p_wc, \
             tc.tile_pool(name="obuf", bufs=8) as p_ob, \
             tc.tile_pool(name="ps", bufs=4, space="PSUM") as p_ps:

            # ---------------- Phase A: hf8 ----------------
            # critical-path input loads split across the sync+scalar queues
            sTt = p_s.tile([128, KH, TPC], FP8, tag="sT")
            nc.sync.dma_start(sTt[:, :KH // 2, :], d_sT[:, :KH * TPC // 2])
            nc.scalar.dma_start(sTt[:, KH // 2:, :], d_sT[:, KH * TPC // 2:])
            NQ = 8                      # wfc m-pair pieces (m-major packing)
            MQ = KF // NQ               # 2 m per piece
            wfcq = []
            for q in range(NQ):
                t = p_wfc.tile([128, MQ, KH, 128], FP8, tag=f"wfcq{q}")
                eng = nc.scalar if q % 2 else nc.sync
                eng.dma_start(
                    t[:], d_wfc[:, q * MQ * KH * 128:(q + 1) * MQ * KH * 128])
                wfcq.append(t)
            bfc_sb = p_b.tile([128, KF], F32, tag="bfc")
            nc.gpsimd.dma_start(bfc_sb[:], d_bfc[:, :])
            bhp_sb = p_b.tile([128, KE], F32, tag="bhp")
            nc.gpsimd.dma_start(bhp_sb[:], d_bhp[:, :])
            scl_sb = p_b.tile([128, 4], F32, tag="scl")
            nc.gpsimd.dma_start(scl_sb[:], d_scl[:, :])

            # PE warmup: garbage matmuls on a memset tile during the input
            # DMA wait, so HAM is at K=8/8 when phase A starts.
            wz = p_b.tile([128, 512], BF, tag="wz")
            nc.vector.memset(wz[:], 0)
            psw = p_ps.tile([128, 1024], F32, tag="ps")
            for _ in range(12):
                nc.tensor.matmul(psw[:, :512], wz[:, :128], wz[:],
                                 start=True, stop=True)

            h8 = p_hf.tile([128, KF, TPC], FP8, tag="hf")
            for m in range(KF):
                q, ml = divmod(m, MQ)
                ps = p_ps.tile([128, 1024], F32, tag="ps")
                for kp in range(KH // 2):
                    nc.tensor.matmul(
                        ps[:, :512],
                        wfcq[q][:, ml, 2 * kp:2 * kp + 2, :],
                        sTt[:, 2 * kp:2 * kp + 2, :],
                        start=(kp == 0), stop=(kp == KH // 2 - 1),
                        perf_mode=DR)
                rl = p_rl.tile([128, TPC], F32, tag="rl")
                nc.vector.tensor_scalar(rl[:], ps[:, :512], bfc_sb[:, m:m + 1],
                                        0.0, mybir.AluOpType.add,
                                        mybir.AluOpType.max)
                nc.scalar.activation(h8[:, m, :], rl[:], AF.Square,
                                     scale=scl_sb[:, SC_ALPHA:SC_ALPHA + 1])

            # ---------------- Phase B: bf8 ----------------
            whpt = p_whp.tile([128, KE, KF, 128], FP8, tag="whp")
            nc.sync.dma_start(whpt[:], d_whp[:, :])
            bf8 = p_bf.tile([128, KE, TPC], FP8, tag="bf8")
            for m in range(KE):
                ps = p_ps.tile([128, 1024], F32, tag="ps")
                for kp in range(KF // 2):
                    nc.tensor.matmul(
                        ps[:, :512],
                        whpt[:, m, 2 * kp:2 * kp + 2, :],
                        h8[:, 2 * kp:2 * kp + 2, :],
                        start=(kp == 0), stop=(kp == KF // 2 - 1),
                        perf_mode=DR)
                nc.vector.tensor_scalar(bf8[:, m, :], ps[:, :512],
                                        bhp_sb[:, m:m + 1],
                                        scl_sb[:, SC_BETA:SC_BETA + 1],
                                        mybir.AluOpType.add,
                                        mybir.AluOpType.mult)

            # wph is needed only by the final C2 phase but must be issued
            # ahead of the ~22MB of Wc chunk traffic on the sync queue
            wpht = p_wph.tile([128, KE, P], FP8, tag="wph")
            nc.sync.dma_start(wpht[:], d_wph[:, :])

            # ---------------- Phase C1: logits (token-major, fp8) -------
            off = 0
            for (b0, csz) in CHUNKS:
                wc = p_wc.tile([128, KE, csz], FP8, tag="wc")
                nc.sync.dma_start(wc[:], d_Wc[:, off:off + KE * csz])
                for tb in range(NT):
                    ob = p_ob.tile([128, csz], FP8, tag="ob")
                    for c0 in range(0, csz, 1024):
                        cw = min(1024, csz - c0)
                        ps = p_ps.tile([128, 1024], F32, tag="ps")
                        for s0 in range(0, cw, 512):
                            w = min(512, cw - s0)
                            for kp in range(KE // 2):
                                nc.tensor.matmul(
                                    ps[:, s0:s0 + w],
                                    bf8[:, 2 * kp:2 * kp + 2,
                                        tb * 128:(tb + 1) * 128],
                                    wc[:, 2 * kp:2 * kp + 2,
                                       c0 + s0:c0 + s0 + w],
                                    start=(kp == 0), stop=(kp == KE // 2 - 1),
                                    perf_mode=DR)
                        drain_mul(ob[:, c0:c0 + cw], ps[:, :cw],
                                  scl_sb[:, SC_OSC1:SC_OSC1 + 1])
                    out_dma(
                        d_outL[tb * 128:(tb + 1) * 128, b0:b0 + csz], ob[:],
                        spread=(csz < CH))
                off += KE * csz
            # ---------------- Phase C2: base partial (token-major) ------
            # runs last: its compute overlaps the outL flush, and its own
            # small outP flush forms the (short) kernel tail
            for tb in range(NT):
                for ph in range(P // 1024):
                    ps = p_ps.tile([128, 1024], F32, tag="ps")
                    for s0 in (0, 512):
                        for kp in range(KE // 2):
                            nc.tensor.matmul(
                                ps[:, s0:s0 + 512],
                                bf8[:, 2 * kp:2 * kp + 2,
                                    tb * 128:(tb + 1) * 128],
                                wpht[:, 2 * kp:2 * kp + 2,
                                     ph * 1024 + s0:ph * 1024 + s0 + 512],
                                start=(kp == 0), stop=(kp == KE // 2 - 1),
                                perf_mode=DR)
                    pb = p_pb.tile([128, 1024], FP8, tag="pb")
                    drain_mul(pb[:], ps[:], scl_sb[:, SC_OSC2:SC_OSC2 + 1])
                    out_dma(
                        d_outP[tb * 128:(tb + 1) * 128,
                               ph * 1024:(ph + 1) * 1024], pb[:], spread=1)

    nc.finalize()
    return nc


def _sigmoid(x):
    return 1.0 / (1.0 + np.exp(-x))


def _host_states_u(inp):
    """Embedding gather, GRU scan, windowed attention -> (states, u, gate)
    all token-major [TOK, ...] with t = s*B + b."""
    f = np.float32
    ids = inp["input_ids"].astype(np.int64)
    emb = inp["emb"].astype(f)

    x = emb[ids]                                     # [B,S,E]
    X = np.ascontiguousarray(x.transpose(1, 0, 2).reshape(TOK, E))
    gi = X @ inp["w_ih"].astype(f).T + inp["b_ih"].astype(f)  # [TOK,3H]

    w_hhT = np.ascontiguousarray(inp["w_hh"].astype(f).T)     # [H,3H]
    b_hh = inp["b_hh"].astype(f)
    h = np.zeros((B, H), f)
    states = np.empty((TOK, H), f)
    for t in range(S):
        hg = h @ w_hhT + b_hh
        gt = gi[t * B:(t + 1) * B]
        r = _sigmoid(gt[:, :H] + hg[:, :H])
        z = _sigmoid(gt[:, H:2 * H] + hg[:, H:2 * H])
        n = np.tanh(gt[:, 2 * H:] + r * hg[:, 2 * H:])
        h = (1.0 - z) * n + z * h
        states[t * B:(t + 1) * B] = h

    q = states @ inp["wq"].astype(f).T + inp["bq"].astype(f)
    k_ = states @ inp["wk"].astype(f).T + inp["bk"].astype(f)
    v_ = states @ inp["wv"].astype(f).T + inp["bv"].astype(f)
    gate = _sigmoid(states @ inp["wg"].astype(f).T + inp["bg"].astype(f))
    ctx = np.zeros((TOK, E), f)
    inv_sqrt = f(1.0 / np.sqrt(MD))
    neg = np.finfo(np.float32).min
    for b in range(B):
        qb, kb, vb = q[b::B], k_[b::B], v_[b::B]
        for i0 in range(0, S, W):
            j0 = max(0, i0 - W)
            sc = (qb[i0:i0 + W] @ kb[j0:i0 + W].T) * inv_sqrt
            i_idx = np.arange(i0, i0 + W)[:, None]
            j_idx = np.arange(j0, i0 + W)[None, :]
            m = (j_idx < i_idx) & (j_idx >= i_idx - W)
            sm = np.where(m, sc, neg)
            sm = sm - sm.max(-1, keepdims=True)
            p_ = np.exp(sm)
            p_ = p_ / p_.sum(-1, keepdims=True)
            p_ = p_ * m
            p_ = p_ / np.clip(p_.sum(-1, keepdims=True), 1e-6, None)
            ctx[i0 * B + b::B][:W] = p_ @ vb[j0:i0 + W]
    g_flat = gate[:, 0] * f(inp["mem_scale"])
    u = g_flat[:, None] * ctx                        # [TOK,E]
    return states, u, g_flat


def _pack_kmaj(arr_T, km, mm):
    """[K*128, M*128] (row k*128+p, col m*128+c) -> [128, (m, k, c)] m-major."""
    return np.ascontiguousarray(
        arr_T.reshape(km, 128, mm, 128).transpose(1, 2, 0, 3).reshape(
            128, mm * km * 128))


def _pack_feat(arr_T, kt, n):
    """[K*128, N] -> [128, (k, n)]."""
    return np.ascontiguousarray(
        arr_T.reshape(kt, 128, n).transpose(1, 0, 2).reshape(128, kt * n))


def kernel(**inputs):
    inp = {k: np.asarray(v) for k, v in inputs.items()}
    f = np.float32
    untied = inp["untied_token_ids"].astype(np.int64)  # [P]
    emb = inp["emb"].astype(f)
    b_ph = inp["b_ph"].astype(f)
    out_bias = inp["out_bias"].astype(f)
    w_ph = inp["w_ph"].astype(f)
    wfc = inp["w_fc"].astype(f)
    whp = inp["w_hp"].astype(f)
    b_fc = inp["b_fc"].astype(f)
    b_hp = inp["b_hp"].astype(f)

    states, u, g_flat = _host_states_u(inp)

    import ml_dtypes
    E4_ = ml_dtypes.float8_e4m3

    # ---- host scale calibration (subsampled fp32 FFN for absmax) ----
    sub = states[::4]
    hf_sub = np.square(np.maximum(sub @ wfc.T + b_fc, 0.0))
    bf_sub = hf_sub @ whp.T + b_hp
    lgmax = np.abs(bf_sub[::8] @ emb.T).max()         # coarse estimate
    bpmax = np.abs(bf_sub[::8] @ w_ph.T).max()        # coarse estimate
    hfmax = hf_sub.max()
    bfmax = np.abs(bf_sub).max()

    ss = M8 / max(np.abs(states).max(), 1e-30)
    swfc = M8 / max(np.abs(wfc).max(), 1e-30)
    rlmax_s = np.sqrt(max(hfmax, 1e-30)) * ss * swfc * 1.15
    salpha = np.sqrt(M8) / rlmax_s
    S_A = (ss * swfc * salpha) ** 2
    swhp = M8 / max(np.abs(whp).max(), 1e-30)
    S_B = S_A * swhp
    beta = M8 / (S_B * max(bfmax, 1e-30) * 1.25)
    sWc = M8 / max(np.abs(emb).max(), 1e-30)
    swph = M8 / max(np.abs(w_ph).max(), 1e-30)
    so = 48.0 / max(lgmax * 2.0, 1e-30)
    osc1 = so / (beta * S_B * sWc)
    so2 = 48.0 / max(bpmax * 2.0, 1e-30)
    osc2 = so2 / (beta * S_B * swph)

    def q8(x, s):
        return np.clip(np.asarray(x, f) * f(s), -240.0, 240.0).astype(E4_)

    # Wc chunk-packed: [128, (chunk, k, col)] matching CHUNKS order
    WcT = q8(np.ascontiguousarray(emb.T), sWc)           # [E, V] fp8
    tmp = WcT.reshape(KE, 128, V).transpose(1, 0, 2)     # [p, k, v]
    Wc_pack = np.empty((128, KE * V), E4_)
    off = 0
    for b0, csz in CHUNKS:
        Wc_pack[:, off:off + KE * csz] = (
            tmp[:, :, b0:b0 + csz].reshape(128, KE * csz))
        off += KE * csz

    scl = np.empty((128, 4), f)
    scl[:, SC_ALPHA] = salpha
    scl[:, SC_BETA] = beta
    scl[:, SC_OSC2] = osc2
    scl[:, SC_OSC1] = osc1

    shared = dict(
        wfc=_pack_kmaj(q8(np.ascontiguousarray(wfc.T), swfc), KH, KF),
        whp=_pack_kmaj(q8(np.ascontiguousarray(whp.T), swhp), KF, KE),
        wph=_pack_feat(q8(np.ascontiguousarray(w_ph.T), swph), KE, P),
        Wc=Wc_pack,
        bfc=np.ascontiguousarray(
            (b_fc * f(ss * swfc)).reshape(KF, 128).T),
        bhp=np.ascontiguousarray(
            (b_hp * f(S_B)).reshape(KE, 128).T),
        scl=scl,
    )
    states8 = q8(states, ss)                             # [TOK, H] fp8
    in_maps = []
    for c in range(NCORES):
        sl = slice(c * TPC, (c + 1) * TPC)
        in_maps.append(dict(
            sT=_pack_feat(np.ascontiguousarray(states8[sl].T), KH, TPC),
            **shared))

    global _last_in_maps
    _last_in_maps = in_maps
    try:
        if not _HAVE_BASS:
            raise RuntimeError("bass toolchain unavailable")
        if "nc" not in _cached:
            _cached["nc"] = _build_program()
        res = run_bass_kernel_spmd(_cached["nc"], in_maps,
                                   core_ids=list(range(NCORES)))
        OT = np.empty((TOK, V), f)
        Pp = np.empty((TOK, P), f)
        inv_so = f(1.0 / so)
        inv_so2 = f(1.0 / so2)
        for c in range(NCORES):
            sl = slice(c * TPC, (c + 1) * TPC)
            OT[sl] = res.results[c]["outL"].astype(f) * inv_so
            Pp[sl] = res.results[c]["outP"].astype(f) * inv_so2
    except Exception as e:
        sys.stderr.write(f"device path failed ({type(e).__name__}: {e}); "
                         "falling back to host compute\n")
        hf = np.square(np.maximum(states @ wfc.T + b_fc, 0.0))
        bf = hf @ whp.T + b_hp
        OT = bf @ emb.T
        Pp = bf @ w_ph.T

    # exact u-term of the partial, in fp32 on host
    Pp += u @ w_ph.T

    if b_ph.any():
        Pp += b_ph[None, :] * (1.0 + g_flat[:, None])
    if out_bias.any():
        OT += out_bias[None, :]

    # scatter-add total_partial into the untied columns (duplicates sum)
    order = np.argsort(untied, kind="stable")
    su = untied[order]
    starts = np.concatenate(([0], np.nonzero(np.diff(su))[0] + 1))
    uids = su[starts]
    seg = np.add.reduceat(Pp[:, order], starts, axis=1)
    OT[:, uids] += seg

    return np.ascontiguousarray(
        OT.reshape(S, B, V).transpose(1, 0, 2))


# revision 35
# speedup vs baseline: 1.1916x; 1.1916x over previous
"""DenseValueWindowedPartialLM kernel for 8 trn2 NeuronCores.

Sharding: token-parallel.  The 4096 tokens (t = s*B + b) are split into 8
contiguous slices of 512; each core computes the FULL 32000-column logits
for its slice.  Per-core device work is exactly total_work/8 (~22 GFLOP).

Precision strategy (validated on host against the fp32 reference):
the final output is dominated by the scattered partial (absmax ~1.5e-2)
while base_logits are tiny (absmax ~4e-5), so the whole GEMM chain runs
in fp8e4m3 with DoubleRow perf mode (2x PE rate) and host-computed
per-tensor scales:
  A: hf = relu(wfc@s)^2, B: bf = whp@hf,
  C1: logits = bf.T@Wc, C2: base_partial = bf.T@wph.
The precision-critical u-term of the partial, u@wph.T (u = gate *
mem_scale * ctx is host-known), is added exactly on the host in fp32,
so device fp8 only ever touches the tiny base quantities.  Both device
outputs are written as scaled fp8, halving the output DMA.

The GRU scan (sequential, 2048 steps) and the tiny windowed attention
run on host; the untied-token scatter-add of the partial logits is a
vectorized host post-pass.

Perf notes (trace-driven; the PE runs gap-free at the fp8 ingest floor
of ~216ns per 512-column DoubleRow matmul):
- vocab is processed in 4096-column chunks (one 2MB weight DMA + 4
  output DMAs per chunk) to keep the sync engine's ~0.6us/descriptor
  issue rate off the critical path; chunk sizes taper at the end so the
  final drain+flush tail stays short.
- output DMAs issue from the otherwise idle gpsimd queue (the tail ones
  alternate with sync); C2 runs LAST so its compute overlaps the outL
  flush and only the small fp8 outP flush trails the last matmul.
- all PSUM goes through one [128,1024] bufs=4 pool (8 banks);
  PSUM->SBUF drains alternate between Vector and Scalar, which are the
  co-bottleneck next to the PE.
- critical-path input loads (sT halves, wfc quarters) are split across
  the sync+scalar queues; 12 warmup matmuls ramp the PE clock while
  they land.
"""

import sys

sys.path.insert(0, "/opt/trn_rl_repo")

import numpy as np

try:
    import concourse.bass as bass
    import concourse.bacc as bacc
    import concourse.mybir as mybir
    import concourse.tile as tile
    from concourse.bass_utils import run_bass_kernel_spmd
    _HAVE_BASS = True
except Exception:  # toolchain unavailable -> host fallback only
    _HAVE_BASS = False

B, S, V, E, H, MD, P, W = 2, 2048, 32000, 512, 1024, 256, 4096, 128
FF = 4 * E                 # 2048
NCORES = 8
TOK = B * S                # 4096 tokens, row t = s*B + b
TPC = TOK // NCORES        # 512 tokens per core
KH, KF, KE = H // 128, FF // 128, E // 128   # 8, 16, 4
NT = TPC // 128                              # 4 token blocks
CH = 4096                                    # vocab chunk
# 7x4096 then tapering tail chunks: the final output DMAs are issued at
# ~0.64us each from the gpsimd queue, so taper chunk sizes to keep the
# end-of-kernel drain+DMA tail short.
CHUNKS = [(b0, CH) for b0 in range(0, 7 * CH, CH)] + [
    (28672, 1024), (29696, 1024), (30720, 768), (31488, 512)]
M8 = 192.0                                   # fp8 target absmax (max norm 240)
if _HAVE_BASS:
    BF = mybir.dt.bfloat16
    F32 = mybir.dt.float32
    FP8 = mybir.dt.float8e4
    AF = mybir.ActivationFunctionType
    DR = mybir.MatmulPerfMode.DoubleRow

_cached = {}

# scale-vector column indices
SC_ALPHA, SC_BETA, SC_OSC2, SC_OSC1 = 0, 1, 2, 3


def _build_program():
    """Per-core program, token slice of TPC=512 tokens (N dim of matmuls).

    A  (fp8 DR): hfT[FF,T] = relu(wfc.T @ sT + bfc)^2      (feature-major)
    B  (fp8 DR): bf8[E,T]  = (whp.T @ hfT + bhp') * beta
    C2 (fp8 DR): outP[T,P] = bf8.T @ wph8 * osc2           (token-major out)
    C1 (fp8 DR): outL[T,V] = bf8.T @ Wc8  * osc1 -> fp8    (token-major out)
    """
    nc = bacc.Bacc()
    d_sT = nc.dram_tensor("sT", [128, KH * TPC], FP8, kind="ExternalInput")
    d_wfc = nc.dram_tensor("wfc", [128, KF * KH * 128], FP8,
                           kind="ExternalInput")   # [p, (m, k, c)]
    d_whp = nc.dram_tensor("whp", [128, KE * KF * 128], FP8,
                           kind="ExternalInput")   # [p, (m, k, c)]
    d_wph = nc.dram_tensor("wph", [128, KE * P], FP8,
                           kind="ExternalInput")   # [p, (k, col)]
    d_Wc = nc.dram_tensor("Wc", [128, KE * V], FP8,
                          kind="ExternalInput")    # [p, (chunk, k, col)]
    d_bfc = nc.dram_tensor("bfc", [128, KF], F32, kind="ExternalInput")
    d_bhp = nc.dram_tensor("bhp", [128, KE], F32, kind="ExternalInput")
    d_scl = nc.dram_tensor("scl", [128, 4], F32, kind="ExternalInput")
    d_outL = nc.dram_tensor("outL", [TPC, V], FP8, kind="ExternalOutput")
    d_outP = nc.dram_tensor("outP", [TPC, P], BF, kind="ExternalOutput")

    dcount = 0

    def drain_mul(dst, src, scol):
        nonlocal dcount
        if dcount % 2 == 0:
            nc.vector.tensor_scalar_mul(dst, src, scol)
        else:
            nc.scalar.mul(dst, src, scol)
        dcount += 1

    ocount = 0

    def out_dma(dst, src, spread=0):
        nonlocal ocount
        # tail chunks: spread issues over otherwise-idle queues so the
        # final flush isn't serialized on gpsimd's ~0.64us issue rate
        if spread and ocount % 2:
            eng = nc.sync
        else:
            eng = nc.gpsimd
        ocount += 1
        eng.dma_start(dst, src)

    with tile.TileContext(nc) as tc:
        with tc.tile_pool(name="sa", bufs=1) as p_s, \
             tc.tile_pool(name="wfc", bufs=1) as p_wfc, \
             tc.tile_pool(name="bias", bufs=1) as p_b, \
             tc.tile_pool(name="rl", bufs=3) as p_rl, \
             tc.tile_pool(name="hf", bufs=1) as p_hf, \
             tc.tile_pool(name="whp", bufs=1) as p_whp, \
             tc.tile_pool(name="bfu", bufs=1) as p_bf, \
             tc.tile_pool(name="wph", bufs=1) as p_wph, \
             tc.tile_pool(name="pbuf", bufs=6) as p_pb, \
             tc.tile_pool(name="wc", bufs=3) as p_wc, \
             tc.tile_pool(name="obuf", bufs=8) as p_ob, \
             tc.tile_pool(name="ps", bufs=4, space="PSUM") as p_ps:

            # ---------------- Phase A: hf8 ----------------
            # critical-path input loads split across the sync+scalar queues
            sTt = p_s.tile([128, KH, TPC], FP8, tag="sT")
            nc.sync.dma_start(sTt[:, :KH // 2, :], d_sT[:, :KH * TPC // 2])
            nc.scalar.dma_start(sTt[:, KH // 2:, :], d_sT[:, KH * TPC // 2:])
            NQ = 8                      # wfc m-pair pieces (m-major packing)
            MQ = KF // NQ               # 2 m per piece
            wfcq = []
            for q in range(NQ):
                t = p_wfc.tile([128, MQ, KH, 128], FP8, tag=f"wfcq{q}")
                eng = nc.scalar if q % 2 else nc.sync
                eng.dma_start(
                    t[:], d_wfc[:, q * MQ * KH * 128:(q + 1) * MQ * KH * 128])
                wfcq.append(t)
            bfc_sb = p_b.tile([128, KF], F32, tag="bfc")
            nc.gpsimd.dma_start(bfc_sb[:], d_bfc[:, :])
            bhp_sb = p_b.tile([128, KE], F32, tag="bhp")
            nc.gpsimd.dma_start(bhp_sb[:], d_bhp[:, :])
            scl_sb = p_b.tile([128, 4], F32, tag="scl")
            nc.gpsimd.dma_start(scl_sb[:], d_scl[:, :])

            # PE warmup: garbage matmuls on a memset tile during the input
            # DMA wait, so HAM is at K=8/8 when phase A starts.
            wz = p_b.tile([128, 512], BF, tag="wz")
            nc.vector.memset(wz[:], 0)
            psw = p_ps.tile([128, 1024], F32, tag="ps")
            for _ in range(12):
                nc.tensor.matmul(psw[:, :512], wz[:, :128], wz[:],
                                 start=True, stop=True)

            h8 = p_hf.tile([128, KF, TPC], FP8, tag="hf")
            for m in range(KF):
                q, ml = divmod(m, MQ)
                ps = p_ps.tile([128, 1024], F32, tag="ps")
                for kp in range(KH // 2):
                    nc.tensor.matmul(
                        ps[:, :512],
                        wfcq[q][:, ml, 2 * kp:2 * kp + 2, :],
                        sTt[:, 2 * kp:2 * kp + 2, :],
                        start=(kp == 0), stop=(kp == KH // 2 - 1),
                        perf_mode=DR)
                rl = p_rl.tile([128, TPC], F32, tag="rl")
                nc.vector.tensor_scalar(rl[:], ps[:, :512], bfc_sb[:, m:m + 1],
                                        0.0, mybir.AluOpType.add,
                                        mybir.AluOpType.max)
                nc.scalar.activation(h8[:, m, :], rl[:], AF.Square,
                                     scale=scl_sb[:, SC_ALPHA:SC_ALPHA + 1])

            # ---------------- Phase B: bf8 ----------------
            whpt = p_whp.tile([128, KE, KF, 128], FP8, tag="whp")
            nc.sync.dma_start(whpt[:], d_whp[:, :])
            bf8 = p_bf.tile([128, KE, TPC], FP8, tag="bf8")
            for m in range(KE):
                ps = p_ps.tile([128, 1024], F32, tag="ps")
                for kp in range(KF // 2):
                    nc.tensor.matmul(
                        ps[:, :512],
                        whpt[:, m, 2 * kp:2 * kp + 2, :],
                        h8[:, 2 * kp:2 * kp + 2, :],
                        start=(kp == 0), stop=(kp == KF // 2 - 1),
                        perf_mode=DR)
                nc.vector.tensor_scalar(bf8[:, m, :], ps[:, :512],
                                        bhp_sb[:, m:m + 1],
                                        scl_sb[:, SC_BETA:SC_BETA + 1],
                                        mybir.AluOpType.add,
                                        mybir.AluOpType.mult)

            # wph is needed only by the final C2 phase but must be issued
            # ahead of the ~22MB of Wc chunk traffic on the sync queue
            wpht = p_wph.tile([128, KE, P], FP8, tag="wph")
            nc.sync.dma_start(wpht[:], d_wph[:, :])

            # ---------------- Phase C1: logits (token-major, fp8) -------
            off = 0
            for (b0, csz) in CHUNKS:
                wc = p_wc.tile([128, KE, csz], FP8, tag="wc")
                nc.sync.dma_start(wc[:], d_Wc[:, off:off + KE * csz])
                for tb in range(NT):
                    ob = p_ob.tile([128, csz], FP8, tag="ob")
                    for c0 in range(0, csz, 1024):
                        cw = min(1024, csz - c0)
                        ps = p_ps.tile([128, 1024], F32, tag="ps")
                        for s0 in range(0, cw, 512):
                            w = min(512, cw - s0)
                            for kp in range(KE // 2):
                                nc.tensor.matmul(
                                    ps[:, s0:s0 + w],
                                    bf8[:, 2 * kp:2 * kp + 2,
                                        tb * 128:(tb + 1) * 128],
                                    wc[:, 2 * kp:2 * kp + 2,
                                       c0 + s0:c0 + s0 + w],
                                    start=(kp == 0), stop=(kp == KE // 2 - 1),
                                    perf_mode=DR)
                        drain_mul(ob[:, c0:c0 + cw], ps[:, :cw],
                                  scl_sb[:, SC_OSC1:SC_OSC1 + 1])
                    out_dma(
                        d_outL[tb * 128:(tb + 1) * 128, b0:b0 + csz], ob[:],
                        spread=(csz < CH))
                off += KE * csz
            # ---------------- Phase C2: base partial (token-major) ------
            # runs last: its compute overlaps the outL flush, and its own
            # small outP flush forms the (short) kernel tail
            for tb in range(NT):
                for ph in range(P // 1024):
                    ps = p_ps.tile([128, 1024], F32, tag="ps")
                    for s0 in (0, 512):
                        for kp in range(KE // 2):
                            nc.tensor.matmul(
                                ps[:, s0:s0 + 512],
                                bf8[:, 2 * kp:2 * kp + 2,
                                    tb * 128:(tb + 1) * 128],
                                wpht[:, 2 * kp:2 * kp + 2,
                                     ph * 1024 + s0:ph * 1024 + s0 + 512],
                                start=(kp == 0), stop=(kp == KE // 2 - 1),
                                perf_mode=DR)
                    pb = p_pb.tile([128, 1024], BF, tag="pb")
                    drain_mul(pb[:], ps[:], scl_sb[:, SC_OSC2:SC_OSC2 + 1])
                    out_dma(
                        d_outP[tb * 128:(tb + 1) * 128,
                               ph * 1024:(ph + 1) * 1024], pb[:], spread=1)

    nc.finalize()
    return nc


def _sigmoid(x):
    return 1.0 / (1.0 + np.exp(-x))


def _host_states_u(inp):
    """Embedding gather, GRU scan, windowed attention -> (states, u, gate)
    all token-major [TOK, ...] with t = s*B + b."""
    f = np.float32
    ids = inp["input_ids"].astype(np.int64)
    emb = inp["emb"].astype(f)

    x = emb[ids]                                     # [B,S,E]
    X = np.ascontiguousarray(x.transpose(1, 0, 2).reshape(TOK, E))
    gi = X @ inp["w_ih"].astype(f).T + inp["b_ih"].astype(f)  # [TOK,3H]

    w_hhT = np.ascontiguousarray(inp["w_hh"].astype(f).T)     # [H,3H]
    b_hh = inp["b_hh"].astype(f)
    h = np.zeros((B, H), f)
    states = np.empty((TOK, H), f)
    for t in range(S):
        hg = h @ w_hhT + b_hh
        gt = gi[t * B:(t + 1) * B]
        r = _sigmoid(gt[:, :H] + hg[:, :H])
        z = _sigmoid(gt[:, H:2 * H] + hg[:, H:2 * H])
        n = np.tanh(gt[:, 2 * H:] + r * hg[:, 2 * H:])
        h = (1.0 - z) * n + z * h
        states[t * B:(t + 1) * B] = h

    q = states @ inp["wq"].astype(f).T + inp["bq"].astype(f)
    k_ = states @ inp["wk"].astype(f).T + inp["bk"].astype(f)
    v_ = states @ inp["wv"].astype(f).T + inp["bv"].astype(f)
    gate = _sigmoid(states @ inp["wg"].astype(f).T + inp["bg"].astype(f))
    ctx = np.zeros((TOK, E), f)
    inv_sqrt = f(1.0 / np.sqrt(MD))
    neg = np.finfo(np.float32).min
    for b in range(B):
        qb, kb, vb = q[b::B], k_[b::B], v_[b::B]
        for i0 in range(0, S, W):
            j0 = max(0, i0 - W)
            sc = (qb[i0:i0 + W] @ kb[j0:i0 + W].T) * inv_sqrt
            i_idx = np.arange(i0, i0 + W)[:, None]
            j_idx = np.arange(j0, i0 + W)[None, :]
            m = (j_idx < i_idx) & (j_idx >= i_idx - W)
            sm = np.where(m, sc, neg)
            sm = sm - sm.max(-1, keepdims=True)
            p_ = np.exp(sm)
            p_ = p_ / p_.sum(-1, keepdims=True)
            p_ = p_ * m
            p_ = p_ / np.clip(p_.sum(-1, keepdims=True), 1e-6, None)
            ctx[i0 * B + b::B][:W] = p_ @ vb[j0:i0 + W]
    g_flat = gate[:, 0] * f(inp["mem_scale"])
    u = g_flat[:, None] * ctx                        # [TOK,E]
    return states, u, g_flat


def _pack_kmaj(arr_T, km, mm):
    """[K*128, M*128] (row k*128+p, col m*128+c) -> [128, (m, k, c)] m-major."""
    return np.ascontiguousarray(
        arr_T.reshape(km, 128, mm, 128).transpose(1, 2, 0, 3).reshape(
            128, mm * km * 128))


def _pack_feat(arr_T, kt, n):
    """[K*128, N] -> [128, (k, n)]."""
    return np.ascontiguousarray(
        arr_T.reshape(kt, 128, n).transpose(1, 0, 2).reshape(128, kt * n))


def kernel(**inputs):
    inp = {k: np.asarray(v) for k, v in inputs.items()}
    f = np.float32
    untied = inp["untied_token_ids"].astype(np.int64)  # [P]
    emb = inp["emb"].astype(f)
    b_ph = inp["b_ph"].astype(f)
    out_bias = inp["out_bias"].astype(f)
    w_ph = inp["w_ph"].astype(f)
    wfc = inp["w_fc"].astype(f)
    whp = inp["w_hp"].astype(f)
    b_fc = inp["b_fc"].astype(f)
    b_hp = inp["b_hp"].astype(f)

    states, u, g_flat = _host_states_u(inp)

    import ml_dtypes
    E4_ = ml_dtypes.float8_e4m3

    # ---- host scale calibration (subsampled fp32 FFN for absmax) ----
    sub = states[::4]
    hf_sub = np.square(np.maximum(sub @ wfc.T + b_fc, 0.0))
    bf_sub = hf_sub @ whp.T + b_hp
    lgmax = np.abs(bf_sub[::8] @ emb.T).max()         # coarse estimate
    bpmax = np.abs(bf_sub[::8] @ w_ph.T).max()        # coarse estimate
    hfmax = hf_sub.max()
    bfmax = np.abs(bf_sub).max()

    ss = M8 / max(np.abs(states).max(), 1e-30)
    swfc = M8 / max(np.abs(wfc).max(), 1e-30)
    rlmax_s = np.sqrt(max(hfmax, 1e-30)) * ss * swfc * 1.15
    salpha = np.sqrt(M8) / rlmax_s
    S_A = (ss * swfc * salpha) ** 2
    swhp = M8 / max(np.abs(whp).max(), 1e-30)
    S_B = S_A * swhp
    beta = M8 / (S_B * max(bfmax, 1e-30) * 1.25)
    sWc = M8 / max(np.abs(emb).max(), 1e-30)
    swph = M8 / max(np.abs(w_ph).max(), 1e-30)
    so = 48.0 / max(lgmax * 2.0, 1e-30)
    osc1 = so / (beta * S_B * sWc)
    so2 = 48.0 / max(bpmax * 2.0, 1e-30)
    osc2 = so2 / (beta * S_B * swph)

    def q8(x, s):
        return np.clip(np.asarray(x, f) * f(s), -240.0, 240.0).astype(E4_)

    # Wc chunk-packed: [128, (chunk, k, col)] matching CHUNKS order
    WcT = q8(np.ascontiguousarray(emb.T), sWc)           # [E, V] fp8
    tmp = WcT.reshape(KE, 128, V).transpose(1, 0, 2)     # [p, k, v]
    Wc_pack = np.empty((128, KE * V), E4_)
    off = 0
    for b0, csz in CHUNKS:
        Wc_pack[:, off:off + KE * csz] = (
            tmp[:, :, b0:b0 + csz].reshape(128, KE * csz))
        off += KE * csz

    scl = np.empty((128, 4), f)
    scl[:, SC_ALPHA] = salpha
    scl[:, SC_BETA] = beta
    scl[:, SC_OSC2] = osc2
    scl[:, SC_OSC1] = osc1

    shared = dict(
        wfc=_pack_kmaj(q8(np.ascontiguousarray(wfc.T), swfc), KH, KF),
        whp=_pack_kmaj(q8(np.ascontiguousarray(whp.T), swhp), KF, KE),
        wph=_pack_feat(q8(np.ascontiguousarray(w_ph.T), swph), KE, P),
        Wc=Wc_pack,
        bfc=np.ascontiguousarray(
            (b_fc * f(ss * swfc)).reshape(KF, 128).T),
        bhp=np.ascontiguousarray(
            (b_hp * f(S_B)).reshape(KE, 128).T),
        scl=scl,
    )
    states8 = q8(states, ss)                             # [TOK, H] fp8
    in_maps = []
    for c in range(NCORES):
        sl = slice(c * TPC, (c + 1) * TPC)
        in_maps.append(dict(
            sT=_pack_feat(np.ascontiguousarray(states8[sl].T), KH, TPC),
            **shared))

    global _last_in_maps
    _last_in_maps = in_maps
    try:
        if not _HAVE_BASS:
            raise RuntimeError("bass toolchain unavailable")
        if "nc" not in _cached:
            _cached["nc"] = _build_program()
        res = run_bass_kernel_spmd(_cached["nc"], in_maps,
                                   core_ids=list(range(NCORES)))
        OT = np.empty((TOK, V), f)
        Pp = np.empty((TOK, P), f)
        inv_so = f(1.0 / so)
        inv_so2 = f(1.0 / so2)
        for c in range(NCORES):
            sl = slice(c * TPC, (c + 1) * TPC)
            OT[sl] = res.results[c]["outL"].astype(f) * inv_so
            Pp[sl] = res.results[c]["outP"].astype(f) * inv_so2
    except Exception as e:
        sys.stderr.write(f"device path failed ({type(e).__name__}: {e}); "
                         "falling back to host compute\n")
        hf = np.square(np.maximum(states @ wfc.T + b_fc, 0.0))
        bf = hf @ whp.T + b_hp
        OT = bf @ emb.T
        Pp = bf @ w_ph.T

    # exact u-term of the partial, in fp32 on host
    Pp += u @ w_ph.T

    if b_ph.any():
        Pp += b_ph[None, :] * (1.0 + g_flat[:, None])
    if out_bias.any():
        OT += out_bias[None, :]

    # scatter-add total_partial into the untied columns (duplicates sum)
    order = np.argsort(untied, kind="stable")
    su = untied[order]
    starts = np.concatenate(([0], np.nonzero(np.diff(su))[0] + 1))
    uids = su[starts]
    seg = np.add.reduceat(Pp[:, order], starts, axis=1)
    OT[:, uids] += seg

    return np.ascontiguousarray(
        OT.reshape(S, B, V).transpose(1, 0, 2))


# revision 37
# speedup vs baseline: 1.1922x; 1.0006x over previous
"""DenseValueWindowedPartialLM kernel for 8 trn2 NeuronCores.

Sharding: token-parallel.  The 4096 tokens (t = s*B + b) are split into 8
contiguous slices of 512; each core computes the FULL 32000-column logits
for its slice.  Per-core device work is exactly total_work/8 (~22 GFLOP).

Precision strategy (validated on host against the fp32 reference):
the final output is dominated by the scattered partial (absmax ~1.5e-2)
while base_logits are tiny (absmax ~4e-5), so the whole GEMM chain runs
in fp8e4m3 with DoubleRow perf mode (2x PE rate) and host-computed
per-tensor scales:
  A: hf = relu(wfc@s)^2, B: bf = whp@hf,
  C1: logits = bf.T@Wc, C2: base_partial = bf.T@wph.
The precision-critical u-term of the partial, u@wph.T (u = gate *
mem_scale * ctx is host-known), is added exactly on the host in fp32,
so device fp8 only ever touches the tiny base quantities.  outL is
written as scaled fp8, halving the output DMA; outP stays bf16
(writing it as fp8 reproducibly slowed the whole kernel ~20%, cause
unknown -- do not revisit without re-measuring).

The GRU scan (sequential, 2048 steps) and the tiny windowed attention
run on host; the untied-token scatter-add of the partial logits is a
vectorized host post-pass.

Perf notes (trace-driven; the PE runs gap-free at the fp8 ingest floor
of ~216ns per 512-column DoubleRow matmul):
- vocab is processed in 4096-column chunks (one 2MB weight DMA + 4
  output DMAs per chunk) to keep the sync engine's ~0.6us/descriptor
  issue rate off the critical path; chunk sizes taper at the end so the
  final drain+flush tail stays short.
- output DMAs issue from the otherwise idle gpsimd queue (the tail ones
  alternate with sync); C2 runs LAST so its compute overlaps the outL
  flush and only the small fp8 outP flush trails the last matmul.
- all PSUM goes through one [128,1024] bufs=4 pool (8 banks);
  PSUM->SBUF drains alternate between Vector and Scalar, which are the
  co-bottleneck next to the PE.
- critical-path input loads (sT halves, wfc quarters) are split across
  the sync+scalar queues; 12 warmup matmuls ramp the PE clock while
  they land.
"""

import sys

sys.path.insert(0, "/opt/trn_rl_repo")

import numpy as np

try:
    import concourse.bass as bass
    import concourse.bacc as bacc
    import concourse.mybir as mybir
    import concourse.tile as tile
    from concourse.bass_utils import run_bass_kernel_spmd
    _HAVE_BASS = True
except Exception:  # toolchain unavailable -> host fallback only
    _HAVE_BASS = False

B, S, V, E, H, MD, P, W = 2, 2048, 32000, 512, 1024, 256, 4096, 128
FF = 4 * E                 # 2048
NCORES = 8
TOK = B * S                # 4096 tokens, row t = s*B + b
TPC = TOK // NCORES        # 512 tokens per core
KH, KF, KE = H // 128, FF // 128, E // 128   # 8, 16, 4
NT = TPC // 128                              # 4 token blocks
CH = 4096                                    # vocab chunk
# 7x4096 then tapering tail chunks: the final output DMAs are issued at
# ~0.64us each from the gpsimd queue, so taper chunk sizes to keep the
# end-of-kernel drain+DMA tail short.
CHUNKS = [(b0, CH) for b0 in range(0, 7 * CH, CH)] + [
    (28672, 1024), (29696, 1024), (30720, 768), (31488, 512)]
M8 = 192.0                                   # fp8 target absmax (max norm 240)
if _HAVE_BASS:
    BF = mybir.dt.bfloat16
    F32 = mybir.dt.float32
    FP8 = mybir.dt.float8e4
    AF = mybir.ActivationFunctionType
    DR = mybir.MatmulPerfMode.DoubleRow

_cached = {}

# scale-vector column indices
SC_ALPHA, SC_BETA, SC_OSC2, SC_OSC1 = 0, 1, 2, 3


def _build_program():
    """Per-core program, token slice of TPC=512 tokens (N dim of matmuls).

    A  (fp8 DR): hfT[FF,T] = relu(wfc.T @ sT + bfc)^2      (feature-major)
    B  (fp8 DR): bf8[E,T]  = (whp.T @ hfT + bhp') * beta
    C1 (fp8 DR): outL[T,V] = bf8.T @ Wc8  * osc1 -> fp8    (token-major out)
    C2 (fp8 DR): outP[T,P] = bf8.T @ wph8 * osc2 -> bf16   (token-major out)
    """
    nc = bacc.Bacc()
    d_sT = nc.dram_tensor("sT", [128, KH * TPC], FP8, kind="ExternalInput")
    d_wfc = nc.dram_tensor("wfc", [128, KF * KH * 128], FP8,
                           kind="ExternalInput")   # [p, (m, k, c)]
    d_whp = nc.dram_tensor("whp", [128, KE * KF * 128], FP8,
                           kind="ExternalInput")   # [p, (m, k, c)]
    d_wph = nc.dram_tensor("wph", [128, KE * P], FP8,
                           kind="ExternalInput")   # [p, (k, col)]
    d_Wc = nc.dram_tensor("Wc", [128, KE * V], FP8,
                          kind="ExternalInput")    # [p, (chunk, k, col)]
    d_bfc = nc.dram_tensor("bfc", [128, KF], F32, kind="ExternalInput")
    d_bhp = nc.dram_tensor("bhp", [128, KE], F32, kind="ExternalInput")
    d_scl = nc.dram_tensor("scl", [128, 4], F32, kind="ExternalInput")
    d_outL = nc.dram_tensor("outL", [TPC, V], FP8, kind="ExternalOutput")
    d_outP = nc.dram_tensor("outP", [TPC, P], BF, kind="ExternalOutput")

    dcount = 0

    def drain_mul(dst, src, scol):
        nonlocal dcount
        if dcount % 2 == 0:
            nc.vector.tensor_scalar_mul(dst, src, scol)
        else:
            nc.scalar.mul(dst, src, scol)
        dcount += 1

    ocount = 0

    def out_dma(dst, src, spread=0):
        nonlocal ocount
        # tail chunks: spread issues over otherwise-idle queues so the
        # final flush isn't serialized on gpsimd's ~0.64us issue rate
        if spread and ocount % 2:
            eng = nc.sync
        else:
            eng = nc.gpsimd
        ocount += 1
        eng.dma_start(dst, src)

    with tile.TileContext(nc) as tc:
        with tc.tile_pool(name="sa", bufs=1) as p_s, \
             tc.tile_pool(name="wfc", bufs=1) as p_wfc, \
             tc.tile_pool(name="bias", bufs=1) as p_b, \
             tc.tile_pool(name="rl", bufs=3) as p_rl, \
             tc.tile_pool(name="hf", bufs=1) as p_hf, \
             tc.tile_pool(name="whp", bufs=1) as p_whp, \
             tc.tile_pool(name="bfu", bufs=1) as p_bf, \
             tc.tile_pool(name="wph", bufs=1) as p_wph, \
             tc.tile_pool(name="pbuf", bufs=6) as p_pb, \
             tc.tile_pool(name="wc", bufs=3) as p_wc, \
             tc.tile_pool(name="obuf", bufs=8) as p_ob, \
             tc.tile_pool(name="ps", bufs=4, space="PSUM") as p_ps:

            # ---------------- Phase A: hf8 ----------------
            # critical-path input loads split across the sync+scalar queues
            sTt = p_s.tile([128, KH, TPC], FP8, tag="sT")
            nc.sync.dma_start(sTt[:, :KH // 2, :], d_sT[:, :KH * TPC // 2])
            nc.scalar.dma_start(sTt[:, KH // 2:, :], d_sT[:, KH * TPC // 2:])
            NQ = 8                      # wfc m-pair pieces (m-major packing)
            MQ = KF // NQ               # 2 m per piece
            wfcq = []
            for q in range(NQ):
                t = p_wfc.tile([128, MQ, KH, 128], FP8, tag=f"wfcq{q}")
                eng = nc.scalar if q % 2 else nc.sync
                eng.dma_start(
                    t[:], d_wfc[:, q * MQ * KH * 128:(q + 1) * MQ * KH * 128])
                wfcq.append(t)
            bfc_sb = p_b.tile([128, KF], F32, tag="bfc")
            nc.gpsimd.dma_start(bfc_sb[:], d_bfc[:, :])
            bhp_sb = p_b.tile([128, KE], F32, tag="bhp")
            nc.gpsimd.dma_start(bhp_sb[:], d_bhp[:, :])
            scl_sb = p_b.tile([128, 4], F32, tag="scl")
            nc.gpsimd.dma_start(scl_sb[:], d_scl[:, :])

            # PE warmup: garbage matmuls on a memset tile during the input
            # DMA wait, so HAM is at K=8/8 when phase A starts.
            wz = p_b.tile([128, 512], BF, tag="wz")
            nc.vector.memset(wz[:], 0)
            psw = p_ps.tile([128, 1024], F32, tag="ps")
            for _ in range(12):
                nc.tensor.matmul(psw[:, :512], wz[:, :128], wz[:],
                                 start=True, stop=True)

            h8 = p_hf.tile([128, KF, TPC], FP8, tag="hf")
            for m in range(KF):
                q, ml = divmod(m, MQ)
                ps = p_ps.tile([128, 1024], F32, tag="ps")
                for kp in range(KH // 2):
                    nc.tensor.matmul(
                        ps[:, :512],
                        wfcq[q][:, ml, 2 * kp:2 * kp + 2, :],
                        sTt[:, 2 * kp:2 * kp + 2, :],
                        start=(kp == 0), stop=(kp == KH // 2 - 1),
                        perf_mode=DR)
                rl = p_rl.tile([128, TPC], F32, tag="rl")
                nc.vector.tensor_scalar(rl[:], ps[:, :512], bfc_sb[:, m:m + 1],
                                        0.0, mybir.AluOpType.add,
                                        mybir.AluOpType.max)
                nc.scalar.activation(h8[:, m, :], rl[:], AF.Square,
                                     scale=scl_sb[:, SC_ALPHA:SC_ALPHA + 1])

            # ---------------- Phase B: bf8 ----------------
            whpt = p_whp.tile([128, KE, KF, 128], FP8, tag="whp")
            nc.sync.dma_start(whpt[:], d_whp[:, :])
            bf8 = p_bf.tile([128, KE, TPC], FP8, tag="bf8")
            for m in range(KE):
                ps = p_ps.tile([128, 1024], F32, tag="ps")
                for kp in range(KF // 2):
                    nc.tensor.matmul(
                        ps[:, :512],
                        whpt[:, m, 2 * kp:2 * kp + 2, :],
                        h8[:, 2 * kp:2 * kp + 2, :],
                        start=(kp == 0), stop=(kp == KF // 2 - 1),
                        perf_mode=DR)
                nc.vector.tensor_scalar(bf8[:, m, :], ps[:, :512],
                                        bhp_sb[:, m:m + 1],
                                        scl_sb[:, SC_BETA:SC_BETA + 1],
                                        mybir.AluOpType.add,
                                        mybir.AluOpType.mult)

            # wph is needed only by the final C2 phase but must be issued
            # ahead of the ~22MB of Wc chunk traffic on the sync queue
            wpht = p_wph.tile([128, KE, P], FP8, tag="wph")
            nc.sync.dma_start(wpht[:], d_wph[:, :])

            # ---------------- Phase C1: logits (token-major, fp8) -------
            off = 0
            for (b0, csz) in CHUNKS:
                wc = p_wc.tile([128, KE, csz], FP8, tag="wc")
                nc.sync.dma_start(wc[:], d_Wc[:, off:off + KE * csz])
                for tb in range(NT):
                    ob = p_ob.tile([128, csz], FP8, tag="ob")
                    for c0 in range(0, csz, 1024):
                        cw = min(1024, csz - c0)
                        ps = p_ps.tile([128, 1024], F32, tag="ps")
                        for s0 in range(0, cw, 512):
                            w = min(512, cw - s0)
                            for kp in range(KE // 2):
                                nc.tensor.matmul(
                                    ps[:, s0:s0 + w],
                                    bf8[:, 2 * kp:2 * kp + 2,
                                        tb * 128:(tb + 1) * 128],
                                    wc[:, 2 * kp:2 * kp + 2,
                                       c0 + s0:c0 + s0 + w],
                                    start=(kp == 0), stop=(kp == KE // 2 - 1),
                                    perf_mode=DR)
                        drain_mul(ob[:, c0:c0 + cw], ps[:, :cw],
                                  scl_sb[:, SC_OSC1:SC_OSC1 + 1])
                    out_dma(
                        d_outL[tb * 128:(tb + 1) * 128, b0:b0 + csz], ob[:],
                        spread=(csz < CH))
                off += KE * csz
            # ---------------- Phase C2: base partial (token-major) ------
            # runs last: its compute overlaps the outL flush, and its own
            # small outP flush forms the (short) kernel tail
            for tb in range(NT):
                for ph in range(P // 1024):
                    ps = p_ps.tile([128, 1024], F32, tag="ps")
                    for s0 in (0, 512):
                        for kp in range(KE // 2):
                            nc.tensor.matmul(
                                ps[:, s0:s0 + 512],
                                bf8[:, 2 * kp:2 * kp + 2,
                                    tb * 128:(tb + 1) * 128],
                                wpht[:, 2 * kp:2 * kp + 2,
                                     ph * 1024 + s0:ph * 1024 + s0 + 512],
                                start=(kp == 0), stop=(kp == KE // 2 - 1),
                                perf_mode=DR)
                    pb = p_pb.tile([128, 1024], BF, tag="pb")
                    drain_mul(pb[:], ps[:], scl_sb[:, SC_OSC2:SC_OSC2 + 1])
                    out_dma(
                        d_outP[tb * 128:(tb + 1) * 128,
                               ph * 1024:(ph + 1) * 1024], pb[:], spread=1)

    nc.finalize()
    return nc


def _sigmoid(x):
    return 1.0 / (1.0 + np.exp(-x))


def _host_states_u(inp):
    """Embedding gather, GRU scan, windowed attention -> (states, u, gate)
    all token-major [TOK, ...] with t = s*B + b."""
    f = np.float32
    ids = inp["input_ids"].astype(np.int64)
    emb = inp["emb"].astype(f)

    x = emb[ids]                                     # [B,S,E]
    X = np.ascontiguousarray(x.transpose(1, 0, 2).reshape(TOK, E))
    gi = X @ inp["w_ih"].astype(f).T + inp["b_ih"].astype(f)  # [TOK,3H]

    w_hhT = np.ascontiguousarray(inp["w_hh"].astype(f).T)     # [H,3H]
    b_hh = inp["b_hh"].astype(f)
    h = np.zeros((B, H), f)
    states = np.empty((TOK, H), f)
    for t in range(S):
        hg = h @ w_hhT + b_hh
        gt = gi[t * B:(t + 1) * B]
        r = _sigmoid(gt[:, :H] + hg[:, :H])
        z = _sigmoid(gt[:, H:2 * H] + hg[:, H:2 * H])
        n = np.tanh(gt[:, 2 * H:] + r * hg[:, 2 * H:])
        h = (1.0 - z) * n + z * h
        states[t * B:(t + 1) * B] = h

    q = states @ inp["wq"].astype(f).T + inp["bq"].astype(f)
    k_ = states @ inp["wk"].astype(f).T + inp["bk"].astype(f)
    v_ = states @ inp["wv"].astype(f).T + inp["bv"].astype(f)
    gate = _sigmoid(states @ inp["wg"].astype(f).T + inp["bg"].astype(f))
    ctx = np.zeros((TOK, E), f)
    inv_sqrt = f(1.0 / np.sqrt(MD))
    neg = np.finfo(np.float32).min
    for b in range(B):
        qb, kb, vb = q[b::B], k_[b::B], v_[b::B]
        for i0 in range(0, S, W):
            j0 = max(0, i0 - W)
            sc = (qb[i0:i0 + W] @ kb[j0:i0 + W].T) * inv_sqrt
            i_idx = np.arange(i0, i0 + W)[:, None]
            j_idx = np.arange(j0, i0 + W)[None, :]
            m = (j_idx < i_idx) & (j_idx >= i_idx - W)
            sm = np.where(m, sc, neg)
            sm = sm - sm.max(-1, keepdims=True)
            p_ = np.exp(sm)
            p_ = p_ / p_.sum(-1, keepdims=True)
            p_ = p_ * m
            p_ = p_ / np.clip(p_.sum(-1, keepdims=True), 1e-6, None)
            ctx[i0 * B + b::B][:W] = p_ @ vb[j0:i0 + W]
    g_flat = gate[:, 0] * f(inp["mem_scale"])
    u = g_flat[:, None] * ctx                        # [TOK,E]
    return states, u, g_flat


def _pack_kmaj(arr_T, km, mm):
    """[K*128, M*128] (row k*128+p, col m*128+c) -> [128, (m, k, c)] m-major."""
    return np.ascontiguousarray(
        arr_T.reshape(km, 128, mm, 128).transpose(1, 2, 0, 3).reshape(
            128, mm * km * 128))


def _pack_feat(arr_T, kt, n):
    """[K*128, N] -> [128, (k, n)]."""
    return np.ascontiguousarray(
        arr_T.reshape(kt, 128, n).transpose(1, 0, 2).reshape(128, kt * n))


def kernel(**inputs):
    inp = {k: np.asarray(v) for k, v in inputs.items()}
    f = np.float32
    untied = inp["untied_token_ids"].astype(np.int64)  # [P]
    emb = inp["emb"].astype(f)
    b_ph = inp["b_ph"].astype(f)
    out_bias = inp["out_bias"].astype(f)
    w_ph = inp["w_ph"].astype(f)
    wfc = inp["w_fc"].astype(f)
    whp = inp["w_hp"].astype(f)
    b_fc = inp["b_fc"].astype(f)
    b_hp = inp["b_hp"].astype(f)

    states, u, g_flat = _host_states_u(inp)

    import ml_dtypes
    E4_ = ml_dtypes.float8_e4m3

    # ---- host scale calibration (subsampled fp32 FFN for absmax) ----
    sub = states[::4]
    hf_sub = np.square(np.maximum(sub @ wfc.T + b_fc, 0.0))
    bf_sub = hf_sub @ whp.T + b_hp
    lgmax = np.abs(bf_sub[::8] @ emb.T).max()         # coarse estimate
    bpmax = np.abs(bf_sub[::8] @ w_ph.T).max()        # coarse estimate
    hfmax = hf_sub.max()
    bfmax = np.abs(bf_sub).max()

    ss = M8 / max(np.abs(states).max(), 1e-30)
    swfc = M8 / max(np.abs(wfc).max(), 1e-30)
    rlmax_s = np.sqrt(max(hfmax, 1e-30)) * ss * swfc * 1.15
    salpha = np.sqrt(M8) / rlmax_s
    S_A = (ss * swfc * salpha) ** 2
    swhp = M8 / max(np.abs(whp).max(), 1e-30)
    S_B = S_A * swhp
    beta = M8 / (S_B * max(bfmax, 1e-30) * 1.25)
    sWc = M8 / max(np.abs(emb).max(), 1e-30)
    swph = M8 / max(np.abs(w_ph).max(), 1e-30)
    so = 48.0 / max(lgmax * 2.0, 1e-30)
    osc1 = so / (beta * S_B * sWc)
    so2 = 48.0 / max(bpmax * 2.0, 1e-30)
    osc2 = so2 / (beta * S_B * swph)

    def q8(x, s):
        return np.clip(np.asarray(x, f) * f(s), -240.0, 240.0).astype(E4_)

    # Wc chunk-packed: [128, (chunk, k, col)] matching CHUNKS order
    WcT = q8(np.ascontiguousarray(emb.T), sWc)           # [E, V] fp8
    tmp = WcT.reshape(KE, 128, V).transpose(1, 0, 2)     # [p, k, v]
    Wc_pack = np.empty((128, KE * V), E4_)
    off = 0
    for b0, csz in CHUNKS:
        Wc_pack[:, off:off + KE * csz] = (
            tmp[:, :, b0:b0 + csz].reshape(128, KE * csz))
        off += KE * csz

    scl = np.empty((128, 4), f)
    scl[:, SC_ALPHA] = salpha
    scl[:, SC_BETA] = beta
    scl[:, SC_OSC2] = osc2
    scl[:, SC_OSC1] = osc1

    shared = dict(
        wfc=_pack_kmaj(q8(np.ascontiguousarray(wfc.T), swfc), KH, KF),
        whp=_pack_kmaj(q8(np.ascontiguousarray(whp.T), swhp), KF, KE),
        wph=_pack_feat(q8(np.ascontiguousarray(w_ph.T), swph), KE, P),
        Wc=Wc_pack,
        bfc=np.ascontiguousarray(
            (b_fc * f(ss * swfc)).reshape(KF, 128).T),
        bhp=np.ascontiguousarray(
            (b_hp * f(S_B)).reshape(KE, 128).T),
        scl=scl,
    )
    states8 = q8(states, ss)                             # [TOK, H] fp8
    in_maps = []
    for c in range(NCORES):
        sl = slice(c * TPC, (c + 1) * TPC)
        in_maps.append(dict(
            sT=_pack_feat(np.ascontiguousarray(states8[sl].T), KH, TPC),
            **shared))

    global _last_in_maps
    _last_in_maps = in_maps
    try:
        if not _HAVE_BASS:
            raise RuntimeError("bass toolchain unavailable")
        if "nc" not in _cached:
            _cached["nc"] = _build_program()
        res = run_bass_kernel_spmd(_cached["nc"], in_maps,
                                   core_ids=list(range(NCORES)))
        OT = np.empty((TOK, V), f)
        Pp = np.empty((TOK, P), f)
        inv_so = f(1.0 / so)
        inv_so2 = f(1.0 / so2)
        for c in range(NCORES):
            sl = slice(c * TPC, (c + 1) * TPC)
            OT[sl] = res.results[c]["outL"].astype(f) * inv_so
            Pp[sl] = res.results[c]["outP"].astype(f) * inv_so2
    except Exception as e:
        sys.stderr.write(f"device path failed ({type(e).__name__}: {e}); "
                         "falling back to host compute\n")
        hf = np.square(np.maximum(states @ wfc.T + b_fc, 0.0))
        bf = hf @ whp.T + b_hp
        OT = bf @ emb.T
        Pp = bf @ w_ph.T

    # exact u-term of the partial, in fp32 on host
    Pp += u @ w_ph.T

    if b_ph.any():
        Pp += b_ph[None, :] * (1.0 + g_flat[:, None])
    if out_bias.any():
        OT += out_bias[None, :]

    # scatter-add total_partial into the untied columns (duplicates sum)
    order = np.argsort(untied, kind="stable")
    su = untied[order]
    starts = np.concatenate(([0], np.nonzero(np.diff(su))[0] + 1))
    uids = su[starts]
    seg = np.add.reduceat(Pp[:, order], starts, axis=1)
    OT[:, uids] += seg

    return np.ascontiguousarray(
        OT.reshape(S, B, V).transpose(1, 0, 2))


# revision 39
# speedup vs baseline: 1.2002x; 1.0067x over previous
"""DenseValueWindowedPartialLM kernel for 8 trn2 NeuronCores.

Sharding: token-parallel.  The 4096 tokens (t = s*B + b) are split into 8
contiguous slices of 512; each core computes the FULL 32000-column logits
for its slice.  Per-core device work is exactly total_work/8 (~22 GFLOP).

Precision strategy (validated on host against the fp32 reference):
the final output is dominated by the scattered partial (absmax ~1.5e-2)
while base_logits are tiny (absmax ~4e-5), so the whole GEMM chain runs
in fp8e4m3 with DoubleRow perf mode (2x PE rate) and host-computed
per-tensor scales:
  A: hf = relu(wfc@s)^2, B: bf = whp@hf,
  C1: logits = bf.T@Wc, C2: base_partial = bf.T@wph.
The precision-critical u-term of the partial, u@wph.T (u = gate *
mem_scale * ctx is host-known), is added exactly on the host in fp32,
so device fp8 only ever touches the tiny base quantities.  outL is
written as scaled fp8, halving the output DMA; outP stays bf16
(writing it as fp8 reproducibly slowed the whole kernel ~20%, cause
unknown -- do not revisit without re-measuring).

The GRU scan (sequential, 2048 steps) and the tiny windowed attention
run on host; the untied-token scatter-add of the partial logits is a
vectorized host post-pass.

Perf notes (trace-driven; the PE runs gap-free at the fp8 ingest floor
of ~216ns per 512-column DoubleRow matmul):
- vocab is processed in 4096-column chunks (one 2MB weight DMA + 4
  output DMAs per chunk) to keep the sync engine's ~0.6us/descriptor
  issue rate off the critical path; chunk sizes taper at the end so the
  final drain+flush tail stays short.
- output DMAs issue from the otherwise idle gpsimd queue (the tail ones
  alternate with sync); C2 runs LAST so its compute overlaps the outL
  flush and only the small fp8 outP flush trails the last matmul.
- all PSUM goes through one [128,1024] bufs=4 pool (8 banks);
  PSUM->SBUF drains alternate between Vector and Scalar, which are the
  co-bottleneck next to the PE.
- critical-path input loads (sT halves, wfc quarters) are split across
  the sync+scalar queues; 12 warmup matmuls ramp the PE clock while
  they land.
"""

import sys

sys.path.insert(0, "/opt/trn_rl_repo")

import numpy as np

try:
    import concourse.bass as bass
    import concourse.bacc as bacc
    import concourse.mybir as mybir
    import concourse.tile as tile
    from concourse.bass_utils import run_bass_kernel_spmd
    _HAVE_BASS = True
except Exception:  # toolchain unavailable -> host fallback only
    _HAVE_BASS = False

B, S, V, E, H, MD, P, W = 2, 2048, 32000, 512, 1024, 256, 4096, 128
FF = 4 * E                 # 2048
NCORES = 8
TOK = B * S                # 4096 tokens, row t = s*B + b
TPC = TOK // NCORES        # 512 tokens per core
KH, KF, KE = H // 128, FF // 128, E // 128   # 8, 16, 4
NT = TPC // 128                              # 4 token blocks
CH = 4096                                    # vocab chunk
# 7x4096 then tapering tail chunks: the final output DMAs are issued at
# ~0.64us each from the gpsimd queue, so taper chunk sizes to keep the
# end-of-kernel drain+DMA tail short.
CHUNKS = [(b0, CH) for b0 in range(0, 7 * CH, CH)] + [
    (28672, 1024), (29696, 1024), (30720, 768), (31488, 512)]
M8 = 192.0                                   # fp8 target absmax (max norm 240)
if _HAVE_BASS:
    BF = mybir.dt.bfloat16
    F32 = mybir.dt.float32
    FP8 = mybir.dt.float8e4
    AF = mybir.ActivationFunctionType
    DR = mybir.MatmulPerfMode.DoubleRow

_cached = {}

# scale-vector column indices
SC_ALPHA, SC_BETA, SC_OSC2, SC_OSC1 = 0, 1, 2, 3


def _build_program():
    """Per-core program, token slice of TPC=512 tokens (N dim of matmuls).

    A  (fp8 DR): hfT[FF,T] = relu(wfc.T @ sT + bfc)^2      (feature-major)
    B  (fp8 DR): bf8[E,T]  = (whp.T @ hfT + bhp') * beta
    C1 (fp8 DR): outL[T,V] = bf8.T @ Wc8  * osc1 -> fp8    (token-major out)
    C2 (fp8 DR): outP[T,P] = bf8.T @ wph8 * osc2 -> bf16   (token-major out)
    """
    nc = bacc.Bacc()
    d_sT = nc.dram_tensor("sT", [128, KH * TPC], FP8, kind="ExternalInput")
    d_wfc = nc.dram_tensor("wfc", [128, KF * KH * 128], FP8,
                           kind="ExternalInput")   # [p, (m, k, c)]
    d_whp = nc.dram_tensor("whp", [128, KE * KF * 128], FP8,
                           kind="ExternalInput")   # [p, (m, k, c)]
    d_wph = nc.dram_tensor("wph", [128, KE * P], FP8,
                           kind="ExternalInput")   # [p, (k, col)]
    d_Wc = nc.dram_tensor("Wc", [128, KE * V], FP8,
                          kind="ExternalInput")    # [p, (chunk, k, col)]
    d_bfc = nc.dram_tensor("bfc", [128, KF], F32, kind="ExternalInput")
    d_bhp = nc.dram_tensor("bhp", [128, KE], F32, kind="ExternalInput")
    d_scl = nc.dram_tensor("scl", [128, 4], F32, kind="ExternalInput")
    d_outL = nc.dram_tensor("outL", [TPC, V], FP8, kind="ExternalOutput")
    d_outP = nc.dram_tensor("outP", [TPC, P], BF, kind="ExternalOutput")

    dcount = 0

    def drain_mul(dst, src, scol):
        nonlocal dcount
        if dcount % 2 == 0:
            nc.vector.tensor_scalar_mul(dst, src, scol)
        else:
            nc.scalar.mul(dst, src, scol)
        dcount += 1

    ocount = 0

    def out_dma(dst, src, spread=0):
        nonlocal ocount
        # tail chunks: spread issues over otherwise-idle queues so the
        # final flush isn't serialized on gpsimd's ~0.64us issue rate
        if spread and ocount % 2:
            eng = nc.sync
        else:
            eng = nc.gpsimd
        ocount += 1
        eng.dma_start(dst, src)

    with tile.TileContext(nc) as tc:
        with tc.tile_pool(name="sa", bufs=1) as p_s, \
             tc.tile_pool(name="wfc", bufs=1) as p_wfc, \
             tc.tile_pool(name="bias", bufs=1) as p_b, \
             tc.tile_pool(name="rl", bufs=3) as p_rl, \
             tc.tile_pool(name="hf", bufs=1) as p_hf, \
             tc.tile_pool(name="whp", bufs=1) as p_whp, \
             tc.tile_pool(name="bfu", bufs=1) as p_bf, \
             tc.tile_pool(name="wph", bufs=1) as p_wph, \
             tc.tile_pool(name="pbuf", bufs=6) as p_pb, \
             tc.tile_pool(name="wc", bufs=4) as p_wc, \
             tc.tile_pool(name="obuf", bufs=8) as p_ob, \
             tc.tile_pool(name="ps", bufs=4, space="PSUM") as p_ps:

            # ---------------- Phase A: hf8 ----------------
            # critical-path input loads split across the sync+scalar queues
            sTt = p_s.tile([128, KH, TPC], FP8, tag="sT")
            nc.sync.dma_start(sTt[:, :KH // 2, :], d_sT[:, :KH * TPC // 2])
            nc.scalar.dma_start(sTt[:, KH // 2:, :], d_sT[:, KH * TPC // 2:])
            NQ = 8                      # wfc m-pair pieces (m-major packing)
            MQ = KF // NQ               # 2 m per piece
            wfcq = []
            for q in range(NQ):
                t = p_wfc.tile([128, MQ, KH, 128], FP8, tag=f"wfcq{q}")
                eng = nc.scalar if q % 2 else nc.sync
                eng.dma_start(
                    t[:], d_wfc[:, q * MQ * KH * 128:(q + 1) * MQ * KH * 128])
                wfcq.append(t)
            bfc_sb = p_b.tile([128, KF], F32, tag="bfc")
            nc.gpsimd.dma_start(bfc_sb[:], d_bfc[:, :])
            bhp_sb = p_b.tile([128, KE], F32, tag="bhp")
            nc.gpsimd.dma_start(bhp_sb[:], d_bhp[:, :])
            scl_sb = p_b.tile([128, 4], F32, tag="scl")
            nc.gpsimd.dma_start(scl_sb[:], d_scl[:, :])

            # PE warmup: garbage matmuls on a memset tile during the input
            # DMA wait, so HAM is at K=8/8 when phase A starts.
            wz = p_b.tile([128, 512], BF, tag="wz")
            nc.vector.memset(wz[:], 0)
            psw = p_ps.tile([128, 1024], F32, tag="ps")
            for _ in range(12):
                nc.tensor.matmul(psw[:, :512], wz[:, :128], wz[:],
                                 start=True, stop=True)

            h8 = p_hf.tile([128, KF, TPC], FP8, tag="hf")
            for m in range(KF):
                q, ml = divmod(m, MQ)
                ps = p_ps.tile([128, 1024], F32, tag="ps")
                for kp in range(KH // 2):
                    nc.tensor.matmul(
                        ps[:, :512],
                        wfcq[q][:, ml, 2 * kp:2 * kp + 2, :],
                        sTt[:, 2 * kp:2 * kp + 2, :],
                        start=(kp == 0), stop=(kp == KH // 2 - 1),
                        perf_mode=DR)
                rl = p_rl.tile([128, TPC], F32, tag="rl")
                nc.vector.tensor_scalar(rl[:], ps[:, :512], bfc_sb[:, m:m + 1],
                                        0.0, mybir.AluOpType.add,
                                        mybir.AluOpType.max)
                nc.scalar.activation(h8[:, m, :], rl[:], AF.Square,
                                     scale=scl_sb[:, SC_ALPHA:SC_ALPHA + 1])

            # ---------------- Phase B: bf8 ----------------
            whpt = p_whp.tile([128, KE, KF, 128], FP8, tag="whp")
            nc.sync.dma_start(whpt[:], d_whp[:, :])
            bf8 = p_bf.tile([128, KE, TPC], FP8, tag="bf8")
            for m in range(KE):
                ps = p_ps.tile([128, 1024], F32, tag="ps")
                for kp in range(KF // 2):
                    nc.tensor.matmul(
                        ps[:, :512],
                        whpt[:, m, 2 * kp:2 * kp + 2, :],
                        h8[:, 2 * kp:2 * kp + 2, :],
                        start=(kp == 0), stop=(kp == KF // 2 - 1),
                        perf_mode=DR)
                nc.vector.tensor_scalar(bf8[:, m, :], ps[:, :512],
                                        bhp_sb[:, m:m + 1],
                                        scl_sb[:, SC_BETA:SC_BETA + 1],
                                        mybir.AluOpType.add,
                                        mybir.AluOpType.mult)

            # wph is needed only by the C2 groups but must be issued ahead
            # of the ~22MB of Wc chunk traffic on the sync queue
            wpht = p_wph.tile([128, KE, P], FP8, tag="wph")
            nc.sync.dma_start(wpht[:], d_wph[:, :])

            def c2_group(tb, ph):
                """One base-partial psum group: outP[tb, ph*1024:+1024]."""
                ps = p_ps.tile([128, 1024], F32, tag="ps")
                for s0 in (0, 512):
                    for kp in range(KE // 2):
                        nc.tensor.matmul(
                            ps[:, s0:s0 + 512],
                            bf8[:, 2 * kp:2 * kp + 2,
                                tb * 128:(tb + 1) * 128],
                            wpht[:, 2 * kp:2 * kp + 2,
                                 ph * 1024 + s0:ph * 1024 + s0 + 512],
                            start=(kp == 0), stop=(kp == KE // 2 - 1),
                            perf_mode=DR)
                pb = p_pb.tile([128, 1024], BF, tag="pb")
                drain_mul(pb[:], ps[:], scl_sb[:, SC_OSC2:SC_OSC2 + 1])
                out_dma(
                    d_outP[tb * 128:(tb + 1) * 128,
                           ph * 1024:(ph + 1) * 1024], pb[:])

            # ------------- Phase C1 + interleaved C2 (token-major) ------
            # The 16 C2 (base partial) groups are spread between mid C1
            # chunks so the 4MB outP flush happens mid-kernel and the
            # kernel ends on C1's small tapered chunks (short tail).
            c2_iter = iter([(tb, ph) for tb in range(NT)
                            for ph in range(P // 1024)])
            off = 0
            for ci, (b0, csz) in enumerate(CHUNKS):
                wc = p_wc.tile([128, KE, csz], FP8, tag="wc")
                nc.sync.dma_start(wc[:], d_Wc[:, off:off + KE * csz])
                for tb in range(NT):
                    ob = p_ob.tile([128, csz], FP8, tag="ob")
                    for c0 in range(0, csz, 1024):
                        cw = min(1024, csz - c0)
                        ps = p_ps.tile([128, 1024], F32, tag="ps")
                        for s0 in range(0, cw, 512):
                            w = min(512, cw - s0)
                            for kp in range(KE // 2):
                                nc.tensor.matmul(
                                    ps[:, s0:s0 + w],
                                    bf8[:, 2 * kp:2 * kp + 2,
                                        tb * 128:(tb + 1) * 128],
                                    wc[:, 2 * kp:2 * kp + 2,
                                       c0 + s0:c0 + s0 + w],
                                    start=(kp == 0), stop=(kp == KE // 2 - 1),
                                    perf_mode=DR)
                        drain_mul(ob[:, c0:c0 + cw], ps[:, :cw],
                                  scl_sb[:, SC_OSC1:SC_OSC1 + 1])
                    out_dma(
                        d_outL[tb * 128:(tb + 1) * 128, b0:b0 + csz], ob[:],
                        spread=(csz < CH))
                off += KE * csz
                if 2 <= ci <= 5:        # 4 C2 groups after chunks 2..5
                    for _ in range(4):
                        nxt = next(c2_iter, None)
                        if nxt is not None:
                            c2_group(*nxt)
            for nxt in c2_iter:         # safety: emit any remainder
                c2_group(*nxt)

    nc.finalize()
    return nc


def _sigmoid(x):
    return 1.0 / (1.0 + np.exp(-x))


def _host_states_u(inp):
    """Embedding gather, GRU scan, windowed attention -> (states, u, gate)
    all token-major [TOK, ...] with t = s*B + b."""
    f = np.float32
    ids = inp["input_ids"].astype(np.int64)
    emb = inp["emb"].astype(f)

    x = emb[ids]                                     # [B,S,E]
    X = np.ascontiguousarray(x.transpose(1, 0, 2).reshape(TOK, E))
    gi = X @ inp["w_ih"].astype(f).T + inp["b_ih"].astype(f)  # [TOK,3H]

    w_hhT = np.ascontiguousarray(inp["w_hh"].astype(f).T)     # [H,3H]
    b_hh = inp["b_hh"].astype(f)
    h = np.zeros((B, H), f)
    states = np.empty((TOK, H), f)
    for t in range(S):
        hg = h @ w_hhT + b_hh
        gt = gi[t * B:(t + 1) * B]
        r = _sigmoid(gt[:, :H] + hg[:, :H])
        z = _sigmoid(gt[:, H:2 * H] + hg[:, H:2 * H])
        n = np.tanh(gt[:, 2 * H:] + r * hg[:, 2 * H:])
        h = (1.0 - z) * n + z * h
        states[t * B:(t + 1) * B] = h

    q = states @ inp["wq"].astype(f).T + inp["bq"].astype(f)
    k_ = states @ inp["wk"].astype(f).T + inp["bk"].astype(f)
    v_ = states @ inp["wv"].astype(f).T + inp["bv"].astype(f)
    gate = _sigmoid(states @ inp["wg"].astype(f).T + inp["bg"].astype(f))
    ctx = np.zeros((TOK, E), f)
    inv_sqrt = f(1.0 / np.sqrt(MD))
    neg = np.finfo(np.float32).min
    for b in range(B):
        qb, kb, vb = q[b::B], k_[b::B], v_[b::B]
        for i0 in range(0, S, W):
            j0 = max(0, i0 - W)
            sc = (qb[i0:i0 + W] @ kb[j0:i0 + W].T) * inv_sqrt
            i_idx = np.arange(i0, i0 + W)[:, None]
            j_idx = np.arange(j0, i0 + W)[None, :]
            m = (j_idx < i_idx) & (j_idx >= i_idx - W)
            sm = np.where(m, sc, neg)
            sm = sm - sm.max(-1, keepdims=True)
            p_ = np.exp(sm)
            p_ = p_ / p_.sum(-1, keepdims=True)
            p_ = p_ * m
            p_ = p_ / np.clip(p_.sum(-1, keepdims=True), 1e-6, None)
            ctx[i0 * B + b::B][:W] = p_ @ vb[j0:i0 + W]
    g_flat = gate[:, 0] * f(inp["mem_scale"])
    u = g_flat[:, None] * ctx                        # [TOK,E]
    return states, u, g_flat


def _pack_kmaj(arr_T, km, mm):
    """[K*128, M*128] (row k*128+p, col m*128+c) -> [128, (m, k, c)] m-major."""
    return np.ascontiguousarray(
        arr_T.reshape(km, 128, mm, 128).transpose(1, 2, 0, 3).reshape(
            128, mm * km * 128))


def _pack_feat(arr_T, kt, n):
    """[K*128, N] -> [128, (k, n)]."""
    return np.ascontiguousarray(
        arr_T.reshape(kt, 128, n).transpose(1, 0, 2).reshape(128, kt * n))


def kernel(**inputs):
    inp = {k: np.asarray(v) for k, v in inputs.items()}
    f = np.float32
    untied = inp["untied_token_ids"].astype(np.int64)  # [P]
    emb = inp["emb"].astype(f)
    b_ph = inp["b_ph"].astype(f)
    out_bias = inp["out_bias"].astype(f)
    w_ph = inp["w_ph"].astype(f)
    wfc = inp["w_fc"].astype(f)
    whp = inp["w_hp"].astype(f)
    b_fc = inp["b_fc"].astype(f)
    b_hp = inp["b_hp"].astype(f)

    states, u, g_flat = _host_states_u(inp)

    import ml_dtypes
    E4_ = ml_dtypes.float8_e4m3

    # ---- host scale calibration (subsampled fp32 FFN for absmax) ----
    sub = states[::4]
    hf_sub = np.square(np.maximum(sub @ wfc.T + b_fc, 0.0))
    bf_sub = hf_sub @ whp.T + b_hp
    lgmax = np.abs(bf_sub[::8] @ emb.T).max()         # coarse estimate
    bpmax = np.abs(bf_sub[::8] @ w_ph.T).max()        # coarse estimate
    hfmax = hf_sub.max()
    bfmax = np.abs(bf_sub).max()

    ss = M8 / max(np.abs(states).max(), 1e-30)
    swfc = M8 / max(np.abs(wfc).max(), 1e-30)
    rlmax_s = np.sqrt(max(hfmax, 1e-30)) * ss * swfc * 1.15
    salpha = np.sqrt(M8) / rlmax_s
    S_A = (ss * swfc * salpha) ** 2
    swhp = M8 / max(np.abs(whp).max(), 1e-30)
    S_B = S_A * swhp
    beta = M8 / (S_B * max(bfmax, 1e-30) * 1.25)
    sWc = M8 / max(np.abs(emb).max(), 1e-30)
    swph = M8 / max(np.abs(w_ph).max(), 1e-30)
    so = 48.0 / max(lgmax * 2.0, 1e-30)
    osc1 = so / (beta * S_B * sWc)
    so2 = 48.0 / max(bpmax * 2.0, 1e-30)
    osc2 = so2 / (beta * S_B * swph)

    def q8(x, s):
        return np.clip(np.asarray(x, f) * f(s), -240.0, 240.0).astype(E4_)

    # Wc chunk-packed: [128, (chunk, k, col)] matching CHUNKS order
    WcT = q8(np.ascontiguousarray(emb.T), sWc)           # [E, V] fp8
    tmp = WcT.reshape(KE, 128, V).transpose(1, 0, 2)     # [p, k, v]
    Wc_pack = np.empty((128, KE * V), E4_)
    off = 0
    for b0, csz in CHUNKS:
        Wc_pack[:, off:off + KE * csz] = (
            tmp[:, :, b0:b0 + csz].reshape(128, KE * csz))
        off += KE * csz

    scl = np.empty((128, 4), f)
    scl[:, SC_ALPHA] = salpha
    scl[:, SC_BETA] = beta
    scl[:, SC_OSC2] = osc2
    scl[:, SC_OSC1] = osc1

    shared = dict(
        wfc=_pack_kmaj(q8(np.ascontiguousarray(wfc.T), swfc), KH, KF),
        whp=_pack_kmaj(q8(np.ascontiguousarray(whp.T), swhp), KF, KE),
        wph=_pack_feat(q8(np.ascontiguousarray(w_ph.T), swph), KE, P),
        Wc=Wc_pack,
        bfc=np.ascontiguousarray(
            (b_fc * f(ss * swfc)).reshape(KF, 128).T),
        bhp=np.ascontiguousarray(
            (b_hp * f(S_B)).reshape(KE, 128).T),
        scl=scl,
    )
    states8 = q8(states, ss)                             # [TOK, H] fp8
    in_maps = []
    for c in range(NCORES):
        sl = slice(c * TPC, (c + 1) * TPC)
        in_maps.append(dict(
            sT=_pack_feat(np.ascontiguousarray(states8[sl].T), KH, TPC),
            **shared))

    global _last_in_maps
    _last_in_maps = in_maps
    try:
        if not _HAVE_BASS:
            raise RuntimeError("bass toolchain unavailable")
        if "nc" not in _cached:
            _cached["nc"] = _build_program()
        res = run_bass_kernel_spmd(_cached["nc"], in_maps,
                                   core_ids=list(range(NCORES)))
        OT = np.empty((TOK, V), f)
        Pp = np.empty((TOK, P), f)
        inv_so = f(1.0 / so)
        inv_so2 = f(1.0 / so2)
        for c in range(NCORES):
            sl = slice(c * TPC, (c + 1) * TPC)
            OT[sl] = res.results[c]["outL"].astype(f) * inv_so
            Pp[sl] = res.results[c]["outP"].astype(f) * inv_so2
    except Exception as e:
        sys.stderr.write(f"device path failed ({type(e).__name__}: {e}); "
                         "falling back to host compute\n")
        hf = np.square(np.maximum(states @ wfc.T + b_fc, 0.0))
        bf = hf @ whp.T + b_hp
        OT = bf @ emb.T
        Pp = bf @ w_ph.T

    # exact u-term of the partial, in fp32 on host
    Pp += u @ w_ph.T

    if b_ph.any():
        Pp += b_ph[None, :] * (1.0 + g_flat[:, None])
    if out_bias.any():
        OT += out_bias[None, :]

    # scatter-add total_partial into the untied columns (duplicates sum)
    order = np.argsort(untied, kind="stable")
    su = untied[order]
    starts = np.concatenate(([0], np.nonzero(np.diff(su))[0] + 1))
    uids = su[starts]
    seg = np.add.reduceat(Pp[:, order], starts, axis=1)
    OT[:, uids] += seg

    return np.ascontiguousarray(
        OT.reshape(S, B, V).transpose(1, 0, 2))
